# revision 22
# baseline (speedup 1.0000x reference)
"""Trainium2 Bass kernel for batched 2-D Gaussian KDE.

reference:
    pdf[b, i] = norm * sum_j exp(-||c_i - c_j||^2 / (2 sigma^2)) * w[b, j]
    with B=8, N=4096, coordinates [B, N, 2], norm = 1/(2 pi sigma^2).

Strategy
--------
Data-parallel over B: one batch element per NeuronCore (8 cores).

Per core, flash-style over j-blocks: the N x N pairwise matrix is never
materialized in DRAM.  The exp argument is produced by a single TensorE
matmul per tile:

    M[i, j] = x_i x_j + y_i y_j + 1 * v_j,   v_j = -|c_j|^2/2 + sigma^2 ln w_j

so that  exp((1/sigma^2) M + bias_i) = norm * w_j * exp(-d2/(2 sigma^2))
with bias_i = -|c_i|^2/(2 sigma^2) + ln norm.

FP32 matmuls run at 1/4 rate on the PE, so each fp32 coordinate is split
exactly into 3 bf16 terms (8-bit mantissa each; 3 terms cover the full 24-bit
fp32 mantissa).  Keeping the 6 product terms >= 2^-27 gives a K=15 bf16
contraction that runs at full PE rate with abs error ~3e-8 on M (1.2e-5 on
the exp argument after the 1/sigma^2 scale).  The splits are computed
ON-DEVICE from f32 x/y/v rows (h = bf16(a), l = bf16(a-h), ll = bf16(a-h-l)
— exact in f32 arithmetic), so the host ships 3 f32 rows per core instead
of 9 bf16 rows: every host-to-device byte costs real wall time through the
relay (~28 ms/MB marginal, measured).

ScalarE evaluates exp in-place on PSUM and its accum_out port emits the
row-sum per 2048-wide tile, so pdf falls out of the activation directly:
no separate reduction pass over the N x N tile is needed.

Dispatch
--------
The devices are axon-tunneled; every synchronous device round trip costs
~90 ms regardless of payload, and re-tracing + re-compiling the jitted
shard_map program costs a further ~190 ms per call.  So the executable is
built exactly once (at import), cached in module globals, and each
kernel() call is a single async dispatch + one blocking fetch.
"""

import sys

sys.path.insert(0, "/opt/trn_rl_repo")

import numpy as np
import ml_dtypes

B = 8
N = 4096
NB = N // 128  # 32 i-blocks of 128
JG = 2048  # j-group width handled by one activation (4 PSUM banks)
NJG = N // JG  # 2
KROWS = 15

_STATE = None
_LAST_RESULT = None


class _Result:
    """Shim matching the attributes test harnesses read."""

    def __init__(self, results):
        self.results = results
        self.exec_time_ns = None
        self.mean_exec_time_ns = None
        self.instructions_and_trace = None
        self.profile_json = None


def _bf16(a):
    return a.astype(ml_dtypes.bfloat16)


def _split3(a64):
    """Exact-ish 3-term bf16 decomposition of a float array (f64 in)."""
    h = _bf16(a64)
    l = _bf16(a64 - h.astype(np.float64))
    ll = _bf16(a64 - h.astype(np.float64) - l.astype(np.float64))
    return h, l, ll


# L/R row sources as (split-term, xyv-row): rows x=0, y=1, v=2
# pairs (i-side, j-side): (h,h) (h,l) (l,h) (h,ll) (ll,h) (l,l) per coord
_L_SRC = [("h",0),("h",0),("l",0),("h",0),("ll",0),("l",0),
          ("h",1),("h",1),("l",1),("h",1),("ll",1),("l",1)]  # rows 12-14 are ones
_R_SRC = [("h",0),("l",0),("h",0),("ll",0),("h",0),("l",0),
          ("h",1),("l",1),("h",1),("ll",1),("h",1),("l",1),
          ("h",2),("l",2),("ll",2)]


def _build(rep=1):
    import contextlib

    import concourse.tile as tile
    from concourse import bacc, mybir

    f32 = mybir.dt.float32
    bf16 = mybir.dt.bfloat16

    nc = bacc.Bacc("TRN2", target_bir_lowering=False, debug=False, num_devices=B)

    XYV_d = nc.dram_tensor("XYV", [3, N], f32, kind="ExternalInput").ap()
    bias_d = nc.dram_tensor("bias", [128, NB], f32, kind="ExternalInput").ap()
    scale_d = nc.dram_tensor("scale", [128, 1], f32, kind="ExternalInput").ap()
    out_d = nc.dram_tensor("out", [128, NB], f32, kind="ExternalOutput").ap()

    with tile.TileContext(nc) as tc:
        with (
            tc.tile_pool(name="sbuf", bufs=1) as pool,
            tc.tile_pool(name="psum", bufs=2, space="PSUM") as psum,
        ):
            L_sb = pool.tile([KROWS, N], bf16)
            R_sb = pool.tile([KROWS, N], bf16)
            xyv_sb = pool.tile([3, N], f32)
            Ch = pool.tile([3, N], bf16)
            Cl = pool.tile([3, N], bf16)
            Cll = pool.tile([3, N], bf16)
            hf = pool.tile([3, N], f32)
            d1 = pool.tile([3, N], f32)
            d2 = pool.tile([3, N], f32)
            bias_sb = pool.tile([128, NB], f32)
            scale_sb = pool.tile([128, 1], f32)
            parts = pool.tile([128, NB * NJG], f32)
            final = pool.tile([128, NB], f32)

            nc.sync.dma_start(xyv_sb[:], XYV_d[:])
            nc.sync.dma_start(bias_sb[:], bias_d[:])
            nc.sync.dma_start(scale_sb[:], scale_d[:])

            # on-device 3-term bf16 split of the x/y/v f32 rows:
            # h = bf16(a); l = bf16(a - h); ll = bf16(a - h - l)
            nc.vector.tensor_copy(Ch[:], xyv_sb[:])
            nc.vector.tensor_copy(hf[:], Ch[:])
            nc.vector.tensor_sub(d1[:], xyv_sb[:], hf[:])
            nc.vector.tensor_copy(Cl[:], d1[:])
            nc.vector.tensor_copy(hf[:], Cl[:])
            nc.vector.tensor_sub(d2[:], d1[:], hf[:])
            nc.vector.tensor_copy(Cll[:], d2[:])

            # rows 12-14 stay at 1.0 (engine partition ranges must start
            # 32-aligned, so memset the whole tile then overwrite 0-11)
            nc.vector.memset(L_sb[:], 1.0)
            split_tiles = {"h": Ch, "l": Cl, "ll": Cll}
            for k, (term, row) in enumerate(_L_SRC):
                t = split_tiles[term]
                nc.sync.dma_start(L_sb[k : k + 1, :], t[row : row + 1, :])
            for k, (term, row) in enumerate(_R_SRC):
                t = split_tiles[term]
                nc.sync.dma_start(R_sb[k : k + 1, :], t[row : row + 1, :])

            loop = tc.For_i(0, rep, 1) if rep > 1 else contextlib.nullcontext()
            with loop:
                for ib in range(NB):
                    lhs = L_sb[:, ib * 128 : (ib + 1) * 128]
                    for g in range(NJG):
                        ps = psum.tile([128, JG], f32)
                        for s in range(JG // 512):
                            j0 = g * JG + s * 512
                            nc.tensor.matmul(
                                ps[:, s * 512 : (s + 1) * 512],
                                lhs,
                                R_sb[:, j0 : j0 + 512],
                                start=True,
                                stop=True,
                            )
                        col = ib * NJG + g
                        nc.scalar.activation(
                            ps[:],
                            ps[:],
                            mybir.ActivationFunctionType.Exp,
                            bias=bias_sb[:, ib : ib + 1],
                            scale=scale_sb[:, 0:1],
                            accum_out=parts[:, col : col + 1],
                        )

                nc.vector.reduce_sum(
                    final[:],
                    parts[:].rearrange("p (a b) -> p a b", b=NJG),
                    axis=mybir.AxisListType.X,
                )
                nc.sync.dma_start(out_d[:], final[:])

    nc.compile()
    return nc


def _prep_all(coordinates, weights, sigma):
    """Host-side prep for all B cores -> concat-ready arrays keyed by name.

    Concat layout matches shard_map's axis-0 sharding: core b owns rows
    [b*rows : (b+1)*rows] of each array.
    """
    bsz = coordinates.shape[0]
    x = coordinates[:, :, 0].astype(np.float64)  # [bsz, N]
    y = coordinates[:, :, 1].astype(np.float64)
    w64 = np.maximum(weights.astype(np.float64), 1e-35)
    sig2 = float(sigma) ** 2
    c = 1.0 / (2.0 * sig2)
    lognorm = -np.log(2.0 * np.pi * sig2)
    sq = x * x + y * y
    v = -0.5 * sq + sig2 * np.log(w64)

    XYV = (
        np.stack([x, y, v], axis=1)
        .reshape(bsz * 3, N)
        .astype(np.float32)
    )
    bias = (
        (-c * sq + lognorm)
        .astype(np.float32)
        .reshape(bsz, NB, 128)
        .transpose(0, 2, 1)
        .reshape(bsz * 128, NB)
    )
    scale = np.full((bsz * 128, 1), 1.0 / sig2, dtype=np.float32)
    return {"XYV": XYV, "bias": bias, "scale": scale}


def _prep_core(xy, w, sigma):
    """Single-core variant (kept for harness/debug compatibility)."""
    full = _prep_all(xy[None], w[None], sigma)
    return {"XYV": full["XYV"][:3], "bias": full["bias"][:128], "scale": full["scale"][:128]}


def _setup():
    """Build the bass program and a cached jitted shard_map executable.

    Mirrors concourse.bass2jax.run_bass_via_pjrt, but constructs the jit
    wrapper once so repeat calls skip tracing/XLA compilation entirely.
    """
    import jax
    from jax.sharding import Mesh, PartitionSpec

    from jax.experimental.shard_map import shard_map

    from concourse import mybir
    from concourse.bass2jax import (
        _bass_exec_p,
        install_neuronx_cc_hook,
        partition_id_tensor,
    )

    nc = _build()
    install_neuronx_cc_hook()

    partition_name = nc.partition_id_tensor.name if nc.partition_id_tensor else None
    in_names = []
    out_names = []
    out_avals = []
    out_shapes = []
    for alloc in nc.m.functions[0].allocations:
        if not isinstance(alloc, mybir.MemoryLocationSet):
            continue
        name = alloc.memorylocations[0].name
        if alloc.kind == "ExternalInput":
            if name != partition_name:
                in_names.append(name)
        elif alloc.kind == "ExternalOutput":
            out_names.append(name)
            shape = tuple(alloc.tensor_shape)
            dtype = mybir.dt.np(alloc.dtype)
            out_avals.append(jax.core.ShapedArray(shape, dtype))
            out_shapes.append((shape, dtype))
    n_params = len(in_names)
    bind_in_names = in_names + out_names
    if partition_name is not None:
        bind_in_names.append(partition_name)
    donate = tuple(range(n_params, n_params + len(out_avals)))

    def _body(*args):
        operands = list(args)
        if partition_name is not None:
            operands.append(partition_id_tensor())
        outs = _bass_exec_p.bind(
            *operands,
            out_avals=tuple(out_avals),
            in_names=tuple(bind_in_names),
            out_names=tuple(out_names),
            lowering_input_output_aliases=(),
            sim_require_finite=True,
            sim_require_nnan=True,
            nc=nc,
        )
        return tuple(outs)

    devices = jax.devices()[:B]
    mesh = Mesh(np.asarray(devices), ("core",))
    in_specs = (PartitionSpec("core"),) * (n_params + len(out_avals))
    out_specs = (PartitionSpec("core"),) * len(out_names)
    fn = jax.jit(
        shard_map(
            _body, mesh=mesh, in_specs=in_specs, out_specs=out_specs, check_rep=False
        ),
        donate_argnums=donate,
        keep_unused=True,
    )

    from jax.sharding import NamedSharding

    return {
        "nc": nc,
        "fn": fn,
        "in_names": in_names,
        "out_names": out_names,
        "out_shapes": out_shapes,
        "sharding": NamedSharding(mesh, PartitionSpec("core")),
        "jax": jax,
    }


def _stage_inputs(state, full):
    """device_put the prepared arrays (async) and retain the device copies."""
    jax = state["jax"]
    sharding = state["sharding"]
    return [
        jax.device_put(np.ascontiguousarray(full[name]), sharding)
        for name in state["in_names"]
    ]


def _run(state, dev_in):
    """One dispatch across the 8 cores; returns concat host outputs.

    The NEFF writes every element of each output, so the donated output
    buffers are never read: recycle the previous call's device-resident
    outputs as this call's donation instead of uploading fresh zeros.
    """
    donor = state.get("recycle")
    if donor is None:
        donor = [
            np.zeros((B * shape[0], *shape[1:]), dtype)
            for shape, dtype in state["out_shapes"]
        ]
    state["recycle"] = None
    try:
        out_arrs = state["fn"](*dev_in, *donor)
        for a in out_arrs:
            try:
                a.copy_to_host_async()
            except Exception:
                pass
        host = [np.asarray(a) for a in out_arrs]
        state["recycle"] = list(out_arrs)
    except Exception:
        state["recycle"] = None
        raise
    return host


def _get_state():
    global _STATE
    if _STATE is None:
        _STATE = _setup()
    return _STATE


def _to_host(*arrs):
    """Convert inputs to numpy, pipelining D2H fetches if they are device
    arrays (three sequential np.asarray calls would each pay a full relay
    round trip; issuing the async copies first coalesces them into one)."""
    for a in arrs:
        f = getattr(a, "copy_to_host_async", None)
        if f is not None:
            try:
                f()
            except Exception:
                pass
    return [np.asarray(a) for a in arrs]


def kernel(weights, coordinates, sigma):
    global _LAST_RESULT
    state = _get_state()

    w, c, sig_arr = _to_host(weights, coordinates, sigma)
    sig = float(sig_arr)

    # Always re-stage inputs: measured A/B showed that reusing cached
    # device-resident inputs makes the call ~20 ms SLOWER — the per-call
    # host-to-device stream primes the relay pipeline, while a bare
    # execute request sits in a batching window.
    full = _prep_all(c, w, sig)
    dev_in = _stage_inputs(state, full)

    host_outs = _run(state, dev_in)
    out = host_outs[0]  # [B*128, NB]
    results = [{"out": out[b * 128 : (b + 1) * 128]} for b in range(B)]
    _LAST_RESULT = _Result(results)

    pdf = (
        out.reshape(B, 128, NB)
        .transpose(0, 2, 1)
        .reshape(B, N)
        .astype(np.float32, copy=False)
    )
    return pdf


def _warmup():
    """Compile + execute at import so the first kernel() call is hot.

    Two runs: the first exercises the numpy-zeros donation signature, the
    second the recycled-device-array signature (distinct pjit cache
    entries — skipping one would leave a retrace for the first real call).
    """
    try:
        state = _get_state()
        rng = np.random.default_rng(0)
        coords = rng.random((B, N, 2), dtype=np.float32)
        w = rng.random((B, N), dtype=np.float32)
        full = _prep_all(coords, w, 0.05)
        dev_in = _stage_inputs(state, full)
        _run(state, dev_in)  # signature: device inputs + numpy-zeros donor
        for _ in range(7):   # recycled-donor signature + settle the relay
            _run(state, dev_in)
    except Exception:
        pass


_warmup()
